# revision 39
# baseline (speedup 1.0000x reference)
"""LoRA multi-head attention on 8 Trainium2 NeuronCores.

Problem: B=4, S=2048, D=1024, H=16, HD=64, RANK=16 LoRA on q/v.
Sharding: core c handles batch c//2 and heads (c%2)*8 .. (c%2)*8+8.
Each (batch, head) is independent through the attention; the out-proj
partial sums (over the two head-halves of a batch) plus the output bias
are reduced on the host during unshard.  No device collectives.

The LoRA adapters are folded into the projection weights on the host
(W_eff = W + scaling * B @ A — exactly equivalent algebra), so the
device kernel runs plain attention.

Per-core dataflow (all matmul inputs bf16, PSUM f32):
  xT[D,S] -> qT/kT[oc,S] (transposed proj, LoRA + 1/sqrt(HD) folded in)
          -> v[S,oc] (natural proj, LoRA folded) with a ones column per head
  scoresT[sk,sq] = kT.T-chunks x qT (2 heads row-tiled in the 128-wide PE)
  expT = Exp(scoresT [+ mask[sk] bias]) on ACT; the all-zero mask case
  dispatches to a no-bias build (measurably faster on ACT)
  ctx[64,sq] per head via COL-TILED concurrent matmul pairs (head A ->
  partitions 0-63, head B -> 64-127 natively; measured 1.86x vs the
  serial 65-row variant).  Softmax denominators come from a bf16
  running sum of the exp tiles on the DVE + two column-sum matmuls per
  iteration (ones-column stationary osB).
  epilogue: ctx psum drained to SBUF at once (frees the accumulation
  banks early), NR-reciprocal of the denominators, bf16 rows, PE K=1
  bf16 broadcasts into each head's partitions, ctxT = ctx * recip
  outT-partial[sq, D] = ctxT-chunks x Wo.T-chunks (ctxT stationary for
  both output halves), copied psum->SBUF then DMA'd to DRAM bf16.
"""

import math
from contextlib import ExitStack

import numpy as np
import ml_dtypes

import concourse.bass as bass
import concourse.mybir as mybir
import concourse.tile as tile
from concourse import bacc
from concourse.bass_utils import run_bass_kernel_spmd

F32 = mybir.dt.float32
BF16 = mybir.dt.bfloat16
NPBF16 = ml_dtypes.bfloat16

B, S, D = 4, 2048, 1024
H, HD = 16, 64
RANK = 16
SCALING = 32.0 / RANK  # 2.0
NCORES = 8
HPC = H // 2        # heads per core = 8
OC = HPC * HD       # output cols per core = 512
NPAIR = HPC // 2    # head pairs per core = 4
KC = D // 128       # 8 contraction chunks
SQB = 512           # sq block
NSQB = S // SQB     # 4
NSK = S // 128      # 16 sk chunks
NSC = S // 128      # 16 s chunks (for v / out-proj)

_NC_CACHE = {}


def _build_nc(loop_n=None, drip_v=True, drip_qk=True, use_mask_bias=True,
              small_out=False):
    # NOTE: GpSimd (Pool) cannot access PSUM on TRN2, so none of the
    # PSUM-draining copies can come off the DVE.
    """Build the (SPMD, per-core) Bass/Tile program once."""
    nc = bacc.Bacc("TRN2", target_bir_lowering=False, debug=False)

    xT_d = nc.dram_tensor("xT", [D, S], BF16, kind="ExternalInput")
    wq_d = nc.dram_tensor("wq", [D, OC], BF16, kind="ExternalInput")
    wk_d = nc.dram_tensor("wk", [D, OC], BF16, kind="ExternalInput")
    wv_d = nc.dram_tensor("wv", [D, OC], BF16, kind="ExternalInput")
    wo_d = nc.dram_tensor("wo", [OC, D], BF16, kind="ExternalInput")
    mask_d = nc.dram_tensor("mask", [128, NSK], F32, kind="ExternalInput")
    # bf16 output halves the 8 MB/core output write traffic; the two
    # per-batch partials are summed in f32 on the host and the quantization
    # adds ~5e-3 relative error, well inside the 2e-2 gate.
    if small_out:
        out_d = nc.dram_tensor("out", [128, 512], BF16, kind="ExternalOutput")
    else:
        out_d = nc.dram_tensor("out", [S, D], BF16, kind="ExternalOutput")

    with tile.TileContext(nc) as tc, ExitStack() as ctx:
        consts = ctx.enter_context(tc.tile_pool(name="consts", bufs=1))
        expp = ctx.enter_context(tc.tile_pool(name="expp", bufs=4))
        dnrp = ctx.enter_context(tc.tile_pool(name="dnrp", bufs=2))
        bcp = ctx.enter_context(tc.tile_pool(name="bcp", bufs=2))
        tmbp = ctx.enter_context(tc.tile_pool(name="tmbp", bufs=2))
        outp = ctx.enter_context(tc.tile_pool(name="outp", bufs=4))
        ps_sc = ctx.enter_context(tc.tile_pool(name="ps_sc", bufs=2, space="PSUM"))
        ps_ctx = ctx.enter_context(tc.tile_pool(name="ps_ctx", bufs=1, space="PSUM"))
        ps_mm = ctx.enter_context(tc.tile_pool(name="ps_mm", bufs=2, space="PSUM"))

        # ---- persistent SBUF tiles --------------------------------------
        xT = consts.tile([128, KC, S], BF16, tag="xT")
        wq = consts.tile([128, KC, OC], BF16, tag="wq")
        wk = consts.tile([128, KC, OC], BF16, tag="wk")
        wv = consts.tile([128, KC, OC], BF16, tag="wv")
        wo = consts.tile([128, NPAIR, D], BF16, tag="wo")
        mask = consts.tile([128, NSK], F32, tag="mask")
        ones = consts.tile([65, 64], BF16, tag="ones")
        # osB col 64 is all-ones, cols 0-63 zero: lhsT osB[:, 64:65] gives a
        # denominator row at psum partition 0; lhsT osB[:, 0:65] gives one at
        # partition 64 (rows 0-63 accumulate zeros)
        osB = consts.tile([128, 65], BF16, tag="osB")
        qT = consts.tile([128, NPAIR, S], BF16, tag="qT")
        kT = consts.tile([128, NPAIR, S], BF16, tag="kT")
        vsb = consts.tile([128, NSK, HPC, HD], BF16, tag="vsb")
        ctxT = consts.tile([128, NPAIR, S], BF16, tag="ctxT")

        def emit():
            # ---- input DMAs -------------------------------------------------
            # interleaved per contraction chunk: the pair-0 k/q ws projection
            # consumes (xT[c], wk[c]) in c order, so chunk-interleaved arrival
            # lets the PE start ~10us earlier than xT-then-weights order.
            # wv last: it is first read by the v drip inside iteration (0,0).
            for c in range(KC):
                nc.sync.dma_start(out=xT[:, c, :], in_=xT_d[c * 128:(c + 1) * 128, :])
                nc.sync.dma_start(out=wk[:, c, :], in_=wk_d[c * 128:(c + 1) * 128, :])
                nc.sync.dma_start(out=wq[:, c, :], in_=wq_d[c * 128:(c + 1) * 128, :])
            for c in range(KC):
                nc.sync.dma_start(out=wv[:, c, :], in_=wv_d[c * 128:(c + 1) * 128, :])
            if use_mask_bias:
                nc.sync.dma_start(out=mask[:, :], in_=mask_d[:, :])
            for p in range(NPAIR):
                nc.sync.dma_start(out=wo[:, p, :], in_=wo_d[p * 128:(p + 1) * 128, :])
            nc.vector.memset(ones[:, :], 1.0)
            nc.vector.memset(osB[:, 0:64], 0.0)
            nc.vector.memset(osB[:, 64:65], 1.0)

            # PE warm-up: the HAM clock gate holds the PE at half rate until
            # ~3.4us of sustained activity.  The input DMAs leave the PE idle
            # at kernel start, so burn that window with tiny matmuls on the
            # ones tile (no input dependency) to enter the loop warm.
            wps = ps_mm.tile([64, 64], F32, tag="ps", name="warm")
            for _ in range(16):
                nc.tensor.matmul(wps[:, :], ones[0:64, :], ones[0:64, :],
                                 start=True, stop=True)

            def emit_v_chunk(sc):
                # v projection for s-chunk sc (natural layout, LoRA folded)
                ps = ps_mm.tile([128, OC], F32, tag="ps")
                for c in range(KC):
                    nc.tensor.matmul(
                        ps[:, :], xT[:, c, sc * 128:(sc + 1) * 128], wv[:, c, :],
                        start=(c == 0), stop=(c == KC - 1))
                nc.vector.tensor_copy(
                    vsb[:, sc, :, :],
                    ps.rearrange("p (h d) -> p h d", h=HPC))

            def emit_qk_proj_ws(p):
                # all 4 sq-blocks of pair p with the weight chunk stationary
                for (wsb, dst) in ((wk, kT), (wq, qT)):
                    pss = [ps_sc.tile([128, SQB], F32, tag="sc", name=f"qps{i}")
                           for i in range(2)] + \
                          [ps_mm.tile([128, SQB], F32, tag="ps", name=f"qpm{i}")
                           for i in range(2)]
                    for c in range(KC):
                        for sqb in range(NSQB):
                            nc.tensor.matmul(
                                pss[sqb][:, :], wsb[:, c, p * 128:(p + 1) * 128],
                                xT[:, c, sqb * SQB:(sqb + 1) * SQB],
                                start=(c == 0), stop=(c == KC - 1))
                    for sqb in range(NSQB):
                        nc.vector.tensor_copy(
                            dst[:, p, sqb * SQB:(sqb + 1) * SQB], pss[sqb][:, :])

            def emit_qk_proj2(p, jp, which):
                # qT or kT rows for pair p, sq blocks 2*jp and 2*jp+1, with
                # the weight chunk stationary across the two blocks, using
                # only the two ps_mm accumulators.
                sqs = [slice((2 * jp + j) * SQB, (2 * jp + j + 1) * SQB)
                       for j in range(2)]
                wsb, dst = (wq, qT) if which == "q" else (wk, kT)
                pss = [ps_mm.tile([128, SQB], F32, tag="ps", name=f"d{j}")
                       for j in range(2)]
                for c in range(KC):
                    for j in range(2):
                        nc.tensor.matmul(
                            pss[j][:, :], wsb[:, c, p * 128:(p + 1) * 128],
                            xT[:, c, sqs[j]],
                            start=(c == 0), stop=(c == KC - 1))
                for j in range(2):
                    nc.vector.tensor_copy(dst[:, p, sqs[j]], pss[j][:, :])

            # pair-0 q/k first so the ACT-bound attention pipeline starts
            # as early as possible; all other PE work (v chunks, later
            # pairs' q/k) is dripped into attention iterations below.
            emit_qk_proj_ws(0)

            # drip-feed schedule: fill[(p, sqb, i)] = list of thunks
            fill = {}
            if drip_v:
                for sc in range(NSC):      # v chunk sc right before first use
                    fill.setdefault((0, 0, sc), []).append(
                        lambda sc=sc: emit_v_chunk(sc))
            else:
                for sc in range(NSC):
                    emit_v_chunk(sc)
            if drip_qk:
                # all later pairs' q/k dripped as EARLY as the dependency
                # structure allows (pair p only has to be ready before
                # attention row p starts).  This releases the xT/wk/wq
                # tiles two attention rows before the iteration ends, so
                # the next loop iteration's input DMAs overlap attention
                # instead of serializing at the loop seam.  Iteration
                # (0,0) is kept clear (it carries the v-projection drip).
                # (Spreading the bundles across later rows to even out
                # per-iteration PE load measured WORSE - the seam overlap
                # matters more than row-level engine balance.)
                slots = [(0, 1, 2), (0, 1, 10), (0, 2, 2), (0, 2, 10),
                         (0, 3, 2), (0, 3, 10), (1, 0, 2), (1, 0, 10),
                         (1, 1, 2), (1, 1, 10), (1, 2, 2), (1, 2, 10)]
                si = 0
                for p in range(1, NPAIR):
                    for (jp, which) in ((0, "k"), (0, "q"), (1, "k"), (1, "q")):
                        blk_p, blk_s, it = slots[si]
                        si += 1
                        fill.setdefault((blk_p, blk_s, it), []).append(
                            lambda p=p, jp=jp, w=which: emit_qk_proj2(p, jp, w))
            else:
                for p in range(1, NPAIR):
                    emit_qk_proj_ws(p)

            def emit_outproj_block(b):
                # pp-outer so the ctxT chunk stays stationary for both output
                # halves (halves the LDWEIGHTS count); both halves accumulate
                # in the two ps_mm banks simultaneously.
                for sc2 in range(b * (SQB // 128), (b + 1) * (SQB // 128)):
                    s2 = slice(sc2 * 128, (sc2 + 1) * 128)
                    pos = [ps_mm.tile([128, 512], F32, tag="ps", name=f"o{oh}")
                           for oh in range(2)]
                    for pp in range(NPAIR):
                        for oh in range(2):
                            nc.tensor.matmul(
                                pos[oh][:, :], ctxT[:, pp, s2],
                                wo[:, pp, oh * 512:(oh + 1) * 512],
                                start=(pp == 0), stop=(pp == NPAIR - 1))
                    for oh in range(2):
                        ot = outp.tile([128, 512], BF16, tag="ot")
                        nc.vector.tensor_copy(ot[:, :], pos[oh][:, :])
                        if small_out:
                            nc.sync.dma_start(out=out_d[:, :], in_=ot[:, :])
                        else:
                            nc.sync.dma_start(
                                out=out_d[s2, oh * 512:(oh + 1) * 512],
                                in_=ot[:, :])

            # ---- per head-pair attention + epilogue ------------------------
            for p in range(NPAIR):
                for sqb in range(NSQB):
                    sq = slice(sqb * SQB, (sqb + 1) * SQB)
                    # one [128, 2*SQB] accumulator: head A in partitions 0-63
                    # of the first bank, head B in partitions 64-127 of the
                    # second (col-tiled ctx matmuls run CONCURRENTLY and head
                    # B lands in its natural partitions - no shift DMA)
                    cc = ps_ctx.tile([128, 2 * SQB], F32, tag="cc")
                    # running bf16 sum of the exp tiles (DVE); its column
                    # sums are the softmax denominators, taken by two PE
                    # matmuls at the end instead of a 65th stationary row
                    # on every chunk
                    sm = dnrp.tile([128, 2 * SQB], BF16, tag="sm")
                    exps = []
                    for i in range(NSK):
                        sk = slice(i * 128, (i + 1) * 128)
                        sc_ps = ps_sc.tile([128, 2 * SQB], F32, tag="sc")
                        # two heads packed in the PE rows (K=64 each)
                        nc.tensor.matmul(
                            sc_ps[:, 0:SQB], kT[0:64, p, sk], qT[0:64, p, sq],
                            start=True, stop=True, tile_position=(0, 0))
                        nc.tensor.matmul(
                            sc_ps[:, SQB:2 * SQB], kT[64:128, p, sk], qT[64:128, p, sq],
                            start=True, stop=True, tile_position=(64, 0))
                        ex = expp.tile([128, 2 * SQB], BF16, tag="ex")
                        nc.scalar.activation(
                            out=ex[:, :], in_=sc_ps[:, :],
                            func=mybir.ActivationFunctionType.Exp,
                            bias=(mask[:, i:i + 1] if use_mask_bias else 0.0),
                            scale=1.0)
                        exps.append(ex)
                        if i == 1:
                            nc.vector.tensor_add(sm[:, :], exps[0][:, :],
                                                 exps[1][:, :])
                        elif i > 1:
                            nc.vector.tensor_add(sm[:, :], sm[:, :],
                                                 exps[i][:, :])
                        for thunk in fill.get((p, sqb, i), ()):
                            thunk()
                        if i > 0:  # ctx of the previous chunk (col-tiled)
                            exl = exps[i - 1]
                            nc.tensor.matmul(
                                cc[0:64, 0:SQB], vsb[:, i - 1, 2 * p, :],
                                exl[:, 0:SQB],
                                start=(i == 1), stop=False,
                                tile_position=(0, 0))
                            nc.tensor.matmul(
                                cc[64:128, SQB:2 * SQB], vsb[:, i - 1, 2 * p + 1, :],
                                exl[:, SQB:2 * SQB],
                                start=(i == 1), stop=False,
                                tile_position=(0, 64))
                    ex = exps[NSK - 1]
                    nc.tensor.matmul(
                        cc[0:64, 0:SQB], vsb[:, NSK - 1, 2 * p, :], ex[:, 0:SQB],
                        start=False, stop=True, tile_position=(0, 0))
                    nc.tensor.matmul(
                        cc[64:128, SQB:2 * SQB], vsb[:, NSK - 1, 2 * p + 1, :],
                        ex[:, SQB:2 * SQB],
                        start=False, stop=True, tile_position=(0, 64))

                    # denominators: column sums of the summed exp tile, head
                    # A at psum partition 0, head B at partition 64 (the
                    # all-zero columns of osB accumulate nothing)
                    dn = ps_mm.tile([65, SQB], F32, tag="ps", name="dn")
                    nc.tensor.matmul(
                        dn[0:1, :], osB[:, 64:65], sm[:, 0:SQB],
                        start=True, stop=False, skip_group_check=True)
                    nc.tensor.matmul(
                        dn[0:65, :], osB[:, 0:65], sm[:, SQB:2 * SQB],
                        start=False, stop=True, skip_group_check=True)

                    # epilogue: drain ctx psum to SBUF (frees the banks),
                    # NR-reciprocal of the denominators (full-tile input;
                    # rows 1-63 are 1/0 garbage nothing reads), bf16 rows,
                    # PE K=1 broadcasts into each head's partitions, scale.
                    cfs = bcp.tile([128, SQB], F32, tag="cfs")
                    nc.vector.tensor_copy(cfs[0:64, :], cc[0:64, 0:SQB])
                    nc.vector.tensor_copy(cfs[64:128, :], cc[64:128, SQB:2 * SQB])
                    dnf = bcp.tile([65, SQB], F32, tag="dnf", name="dnf")
                    nc.vector.tensor_copy(dnf[:, :], dn[:, :])
                    rcp = dnrp.tile([65, SQB], F32, tag="rcp", name="rcp")
                    nc.vector.reciprocal_approx_fast(out=rcp[:, :], in_=dnf[:, :])
                    dnr = dnrp.tile([65, SQB], BF16, tag="dnr", name="dnrb")
                    nc.vector.tensor_copy(dnr[0:1, :], rcp[0:1, :])
                    nc.vector.tensor_copy(dnr[64:65, :], rcp[64:65, :])
                    bcA = ps_mm.tile([64, SQB], F32, tag="ps")
                    nc.tensor.matmul(
                        bcA[:, :], ones[0:1, 0:64], dnr[0:1, :],
                        start=True, stop=True, tile_position=(0, 0))
                    bcBt = ps_mm.tile([128, SQB], F32, tag="ps", name="bcB")
                    nc.tensor.matmul(
                        bcBt[64:128, :], ones[64:65, 0:64], dnr[64:65, :],
                        start=True, stop=True, tile_position=(64, 64))
                    nc.vector.tensor_mul(
                        ctxT[0:64, p, sq], cfs[0:64, :], bcA[:, :])
                    nc.vector.tensor_mul(
                        ctxT[64:128, p, sq], cfs[64:128, :], bcBt[64:128, :])

                    # out-proj of finished sq columns (under last pair)
                    if p == NPAIR - 1:
                        emit_outproj_block(sqb)

        if loop_n is None:
            emit()
        else:
            with tc.For_i(0, loop_n, 1):
                emit()

    nc.compile()
    return nc


def _prep_core_inputs(x, am, Wq, Aq, Bq, Wk, Wv, Av, Bv, Wo):
    """Host-side shard + layout prep. Returns the 8 per-core input dicts.

    The LoRA adapters are folded into the projection weights here:
    x @ W.T + (x @ A.T) @ B.T * s  ==  x @ (W + s * B @ A).T
    so the device kernel runs plain attention.
    """
    s = 1.0 / math.sqrt(HD)
    in_maps = []
    # precompute transposed (LoRA-folded) weight layouts once
    Wq_eff = Wq + SCALING * (Bq @ Aq)
    Wv_eff = Wv + SCALING * (Bv @ Av)
    wqT = np.ascontiguousarray(Wq_eff.T * s).astype(NPBF16)    # [D, D]
    wkT = np.ascontiguousarray(Wk.T).astype(NPBF16)
    wvT = np.ascontiguousarray(Wv_eff.T).astype(NPBF16)
    woT = np.ascontiguousarray(Wo.T).astype(NPBF16)            # [D, D]
    for core in range(NCORES):
        b, hh = core // 2, core % 2
        cs = slice(hh * OC, (hh + 1) * OC)
        xT = np.ascontiguousarray(x[b].T).astype(NPBF16)       # [D, S]
        m = np.ascontiguousarray(
            am[b, 0, 0, :].astype(np.float32).reshape(NSK, 128).T)  # [128, NSK]
        in_maps.append({
            "xT": xT,
            "wq": np.ascontiguousarray(wqT[:, cs]),
            "wk": np.ascontiguousarray(wkT[:, cs]),
            "wv": np.ascontiguousarray(wvT[:, cs]),
            "wo": np.ascontiguousarray(woT[cs, :]),
            "mask": m,
        })
    return in_maps


def kernel(_trace=False, _trace_kwargs=None, **inputs):
    x = np.asarray(inputs["hidden_states"], dtype=np.float32)
    am = np.asarray(inputs["attention_mask"], dtype=np.float32)
    Wq = np.asarray(inputs["Wq"], dtype=np.float32)
    bq = np.asarray(inputs["bq"], dtype=np.float32)
    Aq = np.asarray(inputs["Aq"], dtype=np.float32)
    Bq = np.asarray(inputs["Bq"], dtype=np.float32)
    Wk = np.asarray(inputs["Wk"], dtype=np.float32)
    bk = np.asarray(inputs["bk"], dtype=np.float32)
    Wv = np.asarray(inputs["Wv"], dtype=np.float32)
    bv = np.asarray(inputs["bv"], dtype=np.float32)
    Av = np.asarray(inputs["Av"], dtype=np.float32)
    Bv = np.asarray(inputs["Bv"], dtype=np.float32)
    Wo = np.asarray(inputs["Wo"], dtype=np.float32)
    bo = np.asarray(inputs["bo"], dtype=np.float32)

    # The on-device kernel folds q-scaling into the weights and handles the
    # additive mask; projection biases are all-zero in this problem's
    # regime (asserted here so a violated assumption fails loudly rather
    # than silently returning wrong results).
    assert not bq.any() and not bk.any() and not bv.any(), (
        "non-zero projection biases not supported by this kernel build")

    # The additive mask is all-zeros in this problem's regime; the no-bias
    # exp is measurably faster on the ACT engine, so dispatch on the actual
    # input and keep the bias build as the general fallback.
    key = "nc" if am.any() else "nc_nomask"
    if key not in _NC_CACHE:
        _NC_CACHE[key] = _build_nc(use_mask_bias=(key == "nc"))
    nc = _NC_CACHE[key]

    in_maps = _prep_core_inputs(x, am, Wq, Aq, Bq, Wk, Wv, Av, Bv, Wo)
    res = run_bass_kernel_spmd(
        nc, in_maps, core_ids=list(range(NCORES)), trace=_trace,
        trace_kwargs=_trace_kwargs or {})
    outs = res.results

    out = np.empty((B, S, D), dtype=np.float32)
    for b in range(B):
        out[b] = (outs[2 * b]["out"].astype(np.float32)
                  + outs[2 * b + 1]["out"].astype(np.float32) + bo)
    if _trace:
        return out, res
    return out


# revision 41
# speedup vs baseline: 1.0373x; 1.0373x over previous
"""LoRA multi-head attention on 8 Trainium2 NeuronCores.

Problem: B=4, S=2048, D=1024, H=16, HD=64, RANK=16 LoRA on q/v.
Sharding: core c handles batch c//2 and heads (c%2)*8 .. (c%2)*8+8.
Each (batch, head) is independent through the attention; the out-proj
partial sums (over the two head-halves of a batch) plus the output bias
are reduced on the host during unshard.  No device collectives.

The LoRA adapters are folded into the projection weights on the host
(W_eff = W + scaling * B @ A — exactly equivalent algebra), so the
device kernel runs plain attention.

Per-core dataflow (all matmul inputs bf16, PSUM f32):
  xT[D,S] -> qT/kT[oc,S] (transposed proj, LoRA + 1/sqrt(HD) folded in)
          -> v[S,oc] (natural proj, LoRA folded) with a ones column per head
  scoresT[sk,sq] = kT.T-chunks x qT (2 heads row-tiled in the 128-wide PE)
  expT = Exp(scoresT [+ mask[sk] bias]) on ACT; the all-zero mask case
  dispatches to a no-bias build (measurably faster on ACT)
  ctx[64,sq] per head via COL-TILED concurrent matmul pairs (head A ->
  partitions 0-63, head B -> 64-127 natively; measured 1.86x vs the
  serial 65-row variant).  Softmax denominators come from a bf16
  running sum of the exp tiles on the DVE + two column-sum matmuls per
  iteration (ones-column stationary osB).
  epilogue: ctx psum drained to SBUF at once (frees the accumulation
  banks early), NR-reciprocal of the denominators, bf16 rows, PE K=1
  bf16 broadcasts into each head's partitions, ctxT = ctx * recip
  outT-partial[sq, D] = ctxT-chunks x Wo.T-chunks (ctxT stationary for
  both output halves), copied psum->SBUF then DMA'd to DRAM bf16.
"""

import math
from contextlib import ExitStack

import numpy as np
import ml_dtypes

import concourse.bass as bass
import concourse.mybir as mybir
import concourse.tile as tile
from concourse import bacc
from concourse.bass_utils import run_bass_kernel_spmd

F32 = mybir.dt.float32
BF16 = mybir.dt.bfloat16
NPBF16 = ml_dtypes.bfloat16

B, S, D = 4, 2048, 1024
H, HD = 16, 64
RANK = 16
SCALING = 32.0 / RANK  # 2.0
NCORES = 8
HPC = H // 2        # heads per core = 8
OC = HPC * HD       # output cols per core = 512
NPAIR = HPC // 2    # head pairs per core = 4
KC = D // 128       # 8 contraction chunks
SQB = 512           # sq block
NSQB = S // SQB     # 4
NSK = S // 128      # 16 sk chunks
NSC = S // 128      # 16 s chunks (for v / out-proj)

_NC_CACHE = {}


def _build_nc(loop_n=None, drip_v=True, drip_qk=True, use_mask_bias=True,
              small_out=False):
    # NOTE: GpSimd (Pool) cannot access PSUM on TRN2, so none of the
    # PSUM-draining copies can come off the DVE.
    """Build the (SPMD, per-core) Bass/Tile program once."""
    nc = bacc.Bacc("TRN2", target_bir_lowering=False, debug=False)

    xT_d = nc.dram_tensor("xT", [D, S], BF16, kind="ExternalInput")
    wq_d = nc.dram_tensor("wq", [D, OC], BF16, kind="ExternalInput")
    wk_d = nc.dram_tensor("wk", [D, OC], BF16, kind="ExternalInput")
    wv_d = nc.dram_tensor("wv", [D, OC], BF16, kind="ExternalInput")
    wo_d = nc.dram_tensor("wo", [OC, D], BF16, kind="ExternalInput")
    mask_d = nc.dram_tensor("mask", [128, NSK], F32, kind="ExternalInput")
    # bf16 output halves the 8 MB/core output write traffic; the two
    # per-batch partials are summed in f32 on the host and the quantization
    # adds ~5e-3 relative error, well inside the 2e-2 gate.
    if small_out:
        out_d = nc.dram_tensor("out", [128, 512], BF16, kind="ExternalOutput")
    else:
        out_d = nc.dram_tensor("out", [S, D], BF16, kind="ExternalOutput")

    with tile.TileContext(nc) as tc, ExitStack() as ctx:
        consts = ctx.enter_context(tc.tile_pool(name="consts", bufs=1))
        expp = ctx.enter_context(tc.tile_pool(name="expp", bufs=4))
        dnrp = ctx.enter_context(tc.tile_pool(name="dnrp", bufs=2))
        bcp = ctx.enter_context(tc.tile_pool(name="bcp", bufs=2))
        tmbp = ctx.enter_context(tc.tile_pool(name="tmbp", bufs=2))
        outp = ctx.enter_context(tc.tile_pool(name="outp", bufs=4))
        ps_sc = ctx.enter_context(tc.tile_pool(name="ps_sc", bufs=2, space="PSUM"))
        ps_ctx = ctx.enter_context(tc.tile_pool(name="ps_ctx", bufs=1, space="PSUM"))
        ps_mm = ctx.enter_context(tc.tile_pool(name="ps_mm", bufs=2, space="PSUM"))

        # ---- persistent SBUF tiles --------------------------------------
        xT = consts.tile([128, KC, S], BF16, tag="xT")
        wq = consts.tile([128, KC, OC], BF16, tag="wq")
        wk = consts.tile([128, KC, OC], BF16, tag="wk")
        wv = consts.tile([128, KC, OC], BF16, tag="wv")
        wo = consts.tile([128, NPAIR, D], BF16, tag="wo")
        mask = consts.tile([128, NSK], F32, tag="mask")
        ones = consts.tile([65, 64], BF16, tag="ones")
        # osB col 64 is all-ones, cols 0-63 zero: lhsT osB[:, 64:65] gives a
        # denominator row at psum partition 0; lhsT osB[:, 0:65] gives one at
        # partition 64 (rows 0-63 accumulate zeros)
        osB = consts.tile([128, 65], BF16, tag="osB")
        qT = consts.tile([128, NPAIR, S], BF16, tag="qT")
        kT = consts.tile([128, NPAIR, S], BF16, tag="kT")
        vsb = consts.tile([128, NSK, HPC, HD], BF16, tag="vsb")
        ctxT = consts.tile([128, NPAIR, S], BF16, tag="ctxT")

        def emit():
            # ---- input DMAs -------------------------------------------------
            # interleaved per contraction chunk: the pair-0 k/q ws projection
            # consumes (xT[c], wk[c]) in c order, so chunk-interleaved arrival
            # lets the PE start ~10us earlier than xT-then-weights order.
            # wv last: it is first read by the v drip inside iteration (0,0).
            for c in range(KC):
                nc.sync.dma_start(out=xT[:, c, :], in_=xT_d[c * 128:(c + 1) * 128, :])
                nc.sync.dma_start(out=wk[:, c, :], in_=wk_d[c * 128:(c + 1) * 128, :])
                nc.sync.dma_start(out=wq[:, c, :], in_=wq_d[c * 128:(c + 1) * 128, :])
            for c in range(KC):
                nc.sync.dma_start(out=wv[:, c, :], in_=wv_d[c * 128:(c + 1) * 128, :])
            if use_mask_bias:
                nc.sync.dma_start(out=mask[:, :], in_=mask_d[:, :])
            for p in range(NPAIR):
                nc.sync.dma_start(out=wo[:, p, :], in_=wo_d[p * 128:(p + 1) * 128, :])
            nc.vector.memset(ones[:, :], 1.0)
            nc.vector.memset(osB[:, 0:64], 0.0)
            nc.vector.memset(osB[:, 64:65], 1.0)

            # PE warm-up: the HAM clock gate holds the PE at half rate until
            # ~3.4us of sustained activity.  The input DMAs leave the PE idle
            # at kernel start, so burn that window with tiny matmuls on the
            # ones tile (no input dependency) to enter the loop warm.
            wps = ps_mm.tile([64, 64], F32, tag="ps", name="warm")
            for _ in range(16):
                nc.tensor.matmul(wps[:, :], ones[0:64, :], ones[0:64, :],
                                 start=True, stop=True)

            def emit_v_chunk(sc):
                # v projection for s-chunk sc (natural layout, LoRA folded)
                ps = ps_mm.tile([128, OC], F32, tag="ps")
                for c in range(KC):
                    nc.tensor.matmul(
                        ps[:, :], xT[:, c, sc * 128:(sc + 1) * 128], wv[:, c, :],
                        start=(c == 0), stop=(c == KC - 1))
                nc.vector.tensor_copy(
                    vsb[:, sc, :, :],
                    ps.rearrange("p (h d) -> p h d", h=HPC))

            def emit_qk_proj_ws(p):
                # all 4 sq-blocks of pair p with the weight chunk stationary
                for (wsb, dst) in ((wk, kT), (wq, qT)):
                    pss = [ps_sc.tile([128, SQB], F32, tag="sc", name=f"qps{i}")
                           for i in range(2)] + \
                          [ps_mm.tile([128, SQB], F32, tag="ps", name=f"qpm{i}")
                           for i in range(2)]
                    for c in range(KC):
                        for sqb in range(NSQB):
                            nc.tensor.matmul(
                                pss[sqb][:, :], wsb[:, c, p * 128:(p + 1) * 128],
                                xT[:, c, sqb * SQB:(sqb + 1) * SQB],
                                start=(c == 0), stop=(c == KC - 1))
                    for sqb in range(NSQB):
                        nc.vector.tensor_copy(
                            dst[:, p, sqb * SQB:(sqb + 1) * SQB], pss[sqb][:, :])

            def emit_qk_proj2(p, jp, which):
                # qT or kT rows for pair p, sq blocks 2*jp and 2*jp+1, with
                # the weight chunk stationary across the two blocks, using
                # only the two ps_mm accumulators.
                sqs = [slice((2 * jp + j) * SQB, (2 * jp + j + 1) * SQB)
                       for j in range(2)]
                wsb, dst = (wq, qT) if which == "q" else (wk, kT)
                pss = [ps_mm.tile([128, SQB], F32, tag="ps", name=f"d{j}")
                       for j in range(2)]
                for c in range(KC):
                    for j in range(2):
                        nc.tensor.matmul(
                            pss[j][:, :], wsb[:, c, p * 128:(p + 1) * 128],
                            xT[:, c, sqs[j]],
                            start=(c == 0), stop=(c == KC - 1))
                for j in range(2):
                    nc.vector.tensor_copy(dst[:, p, sqs[j]], pss[j][:, :])

            # pair-0 q/k first so the ACT-bound attention pipeline starts
            # as early as possible; all other PE work (v chunks, later
            # pairs' q/k) is dripped into attention iterations below.
            emit_qk_proj_ws(0)

            # drip-feed schedule: fill[(p, sqb, i)] = list of thunks
            fill = {}
            if drip_v:
                for sc in range(NSC):      # v chunk sc right before first use
                    fill.setdefault((0, 0, sc), []).append(
                        lambda sc=sc: emit_v_chunk(sc))
            else:
                for sc in range(NSC):
                    emit_v_chunk(sc)
            if drip_qk:
                # all later pairs' q/k dripped as EARLY as the dependency
                # structure allows (pair p only has to be ready before
                # attention row p starts).  This releases the xT/wk/wq
                # tiles two attention rows before the iteration ends, so
                # the next loop iteration's input DMAs overlap attention
                # instead of serializing at the loop seam.  Iteration
                # (0,0) is kept clear (it carries the v-projection drip).
                # (Spreading the bundles across later rows to even out
                # per-iteration PE load measured WORSE - the seam overlap
                # matters more than row-level engine balance.)
                slots = [(0, 1, 2), (0, 1, 10), (0, 2, 2), (0, 2, 10),
                         (0, 3, 2), (0, 3, 10), (1, 0, 2), (1, 0, 10),
                         (1, 1, 2), (1, 1, 10), (1, 2, 2), (1, 2, 10)]
                si = 0
                for p in range(1, NPAIR):
                    for (jp, which) in ((0, "k"), (0, "q"), (1, "k"), (1, "q")):
                        blk_p, blk_s, it = slots[si]
                        si += 1
                        fill.setdefault((blk_p, blk_s, it), []).append(
                            lambda p=p, jp=jp, w=which: emit_qk_proj2(p, jp, w))
            else:
                for p in range(1, NPAIR):
                    emit_qk_proj_ws(p)

            def emit_outproj_block(b):
                # pp-outer so the ctxT chunk stays stationary for both output
                # halves (halves the LDWEIGHTS count); both halves accumulate
                # in the two ps_mm banks simultaneously.
                for sc2 in range(b * (SQB // 128), (b + 1) * (SQB // 128)):
                    s2 = slice(sc2 * 128, (sc2 + 1) * 128)
                    pos = [ps_mm.tile([128, 512], F32, tag="ps", name=f"o{oh}")
                           for oh in range(2)]
                    for pp in range(NPAIR):
                        for oh in range(2):
                            nc.tensor.matmul(
                                pos[oh][:, :], ctxT[:, pp, s2],
                                wo[:, pp, oh * 512:(oh + 1) * 512],
                                start=(pp == 0), stop=(pp == NPAIR - 1))
                    for oh in range(2):
                        ot = outp.tile([128, 512], BF16, tag="ot")
                        nc.vector.tensor_copy(ot[:, :], pos[oh][:, :])
                        if small_out:
                            nc.sync.dma_start(out=out_d[:, :], in_=ot[:, :])
                        else:
                            nc.sync.dma_start(
                                out=out_d[s2, oh * 512:(oh + 1) * 512],
                                in_=ot[:, :])

            # ---- per head-pair attention + epilogue ------------------------
            for p in range(NPAIR):
                for sqb in range(NSQB):
                    sq = slice(sqb * SQB, (sqb + 1) * SQB)
                    # one [128, 2*SQB] accumulator: head A in partitions 0-63
                    # of the first bank, head B in partitions 64-127 of the
                    # second (col-tiled ctx matmuls run CONCURRENTLY and head
                    # B lands in its natural partitions - no shift DMA)
                    cc = ps_ctx.tile([128, 2 * SQB], F32, tag="cc")
                    # running bf16 sum of the exp tiles (DVE); its column
                    # sums are the softmax denominators, taken by two PE
                    # matmuls at the end instead of a 65th stationary row
                    # on every chunk
                    sm = dnrp.tile([128, 2 * SQB], BF16, tag="sm")
                    exps = []
                    for i in range(NSK):
                        sk = slice(i * 128, (i + 1) * 128)
                        sc_ps = ps_sc.tile([128, 2 * SQB], F32, tag="sc")
                        # two heads packed in the PE rows (K=64 each)
                        nc.tensor.matmul(
                            sc_ps[:, 0:SQB], kT[0:64, p, sk], qT[0:64, p, sq],
                            start=True, stop=True, tile_position=(0, 0))
                        nc.tensor.matmul(
                            sc_ps[:, SQB:2 * SQB], kT[64:128, p, sk], qT[64:128, p, sq],
                            start=True, stop=True, tile_position=(64, 0))
                        ex = expp.tile([128, 2 * SQB], BF16, tag="ex")
                        nc.scalar.activation(
                            out=ex[:, :], in_=sc_ps[:, :],
                            func=mybir.ActivationFunctionType.Exp,
                            bias=(mask[:, i:i + 1] if use_mask_bias else 0.0),
                            scale=1.0)
                        exps.append(ex)
                        if i == 1:
                            nc.vector.tensor_add(sm[:, :], exps[0][:, :],
                                                 exps[1][:, :])
                        elif i > 1:
                            nc.vector.tensor_add(sm[:, :], sm[:, :],
                                                 exps[i][:, :])
                        for thunk in fill.get((p, sqb, i), ()):
                            thunk()
                        if i > 0:  # ctx of the previous chunk (col-tiled)
                            exl = exps[i - 1]
                            nc.tensor.matmul(
                                cc[0:64, 0:SQB], vsb[:, i - 1, 2 * p, :],
                                exl[:, 0:SQB],
                                start=(i == 1), stop=False,
                                tile_position=(0, 0))
                            nc.tensor.matmul(
                                cc[64:128, SQB:2 * SQB], vsb[:, i - 1, 2 * p + 1, :],
                                exl[:, SQB:2 * SQB],
                                start=(i == 1), stop=False,
                                tile_position=(0, 64))
                    ex = exps[NSK - 1]
                    nc.tensor.matmul(
                        cc[0:64, 0:SQB], vsb[:, NSK - 1, 2 * p, :], ex[:, 0:SQB],
                        start=False, stop=True, tile_position=(0, 0))
                    nc.tensor.matmul(
                        cc[64:128, SQB:2 * SQB], vsb[:, NSK - 1, 2 * p + 1, :],
                        ex[:, SQB:2 * SQB],
                        start=False, stop=True, tile_position=(0, 64))

                    # denominators: column sums of the summed exp tile via
                    # two CONCURRENT M=1 matmuls (col-disjoint array columns
                    # 0 and 64, separate banks); head A lands at partition 0,
                    # head B at partition 64
                    dna = ps_mm.tile([1, SQB], F32, tag="ps", name="dna")
                    nc.tensor.matmul(
                        dna[0:1, :], osB[:, 64:65], sm[:, 0:SQB],
                        start=True, stop=True, tile_position=(0, 0))
                    dnbt = ps_mm.tile([128, SQB], F32, tag="ps", name="dnb")
                    nc.tensor.matmul(
                        dnbt[64:65, :], osB[:, 64:65], sm[:, SQB:2 * SQB],
                        start=True, stop=True, tile_position=(0, 64))

                    # epilogue: drain ctx psum to SBUF (frees the banks),
                    # NR-reciprocal of the denominators (full-tile input;
                    # rows 1-63 are 1/0 garbage nothing reads), bf16 rows,
                    # PE K=1 broadcasts into each head's partitions, scale.
                    cfs = bcp.tile([128, SQB], F32, tag="cfs")
                    nc.vector.tensor_copy(cfs[0:64, :], cc[0:64, 0:SQB])
                    nc.vector.tensor_copy(cfs[64:128, :], cc[64:128, SQB:2 * SQB])
                    dnf = bcp.tile([65, SQB], F32, tag="dnf", name="dnf")
                    nc.vector.tensor_copy(dnf[0:1, :], dna[0:1, :])
                    nc.vector.tensor_copy(dnf[64:65, :], dnbt[64:65, :])
                    rcp = dnrp.tile([65, SQB], F32, tag="rcp", name="rcp")
                    nc.vector.reciprocal_approx_fast(out=rcp[:, :], in_=dnf[:, :])
                    dnr = dnrp.tile([65, SQB], BF16, tag="dnr", name="dnrb")
                    nc.vector.tensor_copy(dnr[0:1, :], rcp[0:1, :])
                    nc.vector.tensor_copy(dnr[64:65, :], rcp[64:65, :])
                    bcA = ps_mm.tile([64, SQB], F32, tag="ps")
                    nc.tensor.matmul(
                        bcA[:, :], ones[0:1, 0:64], dnr[0:1, :],
                        start=True, stop=True, tile_position=(0, 0))
                    bcBt = ps_mm.tile([128, SQB], F32, tag="ps", name="bcB")
                    nc.tensor.matmul(
                        bcBt[64:128, :], ones[64:65, 0:64], dnr[64:65, :],
                        start=True, stop=True, tile_position=(64, 64))
                    nc.vector.tensor_mul(
                        ctxT[0:64, p, sq], cfs[0:64, :], bcA[:, :])
                    nc.vector.tensor_mul(
                        ctxT[64:128, p, sq], cfs[64:128, :], bcBt[64:128, :])

                    # out-proj of finished sq columns (under last pair)
                    if p == NPAIR - 1:
                        emit_outproj_block(sqb)

        if loop_n is None:
            emit()
        else:
            with tc.For_i(0, loop_n, 1):
                emit()

    nc.compile()
    return nc


def _prep_core_inputs(x, am, Wq, Aq, Bq, Wk, Wv, Av, Bv, Wo):
    """Host-side shard + layout prep. Returns the 8 per-core input dicts.

    The LoRA adapters are folded into the projection weights here:
    x @ W.T + (x @ A.T) @ B.T * s  ==  x @ (W + s * B @ A).T
    so the device kernel runs plain attention.
    """
    s = 1.0 / math.sqrt(HD)
    in_maps = []
    # precompute transposed (LoRA-folded) weight layouts once
    Wq_eff = Wq + SCALING * (Bq @ Aq)
    Wv_eff = Wv + SCALING * (Bv @ Av)
    wqT = np.ascontiguousarray(Wq_eff.T * s).astype(NPBF16)    # [D, D]
    wkT = np.ascontiguousarray(Wk.T).astype(NPBF16)
    wvT = np.ascontiguousarray(Wv_eff.T).astype(NPBF16)
    woT = np.ascontiguousarray(Wo.T).astype(NPBF16)            # [D, D]
    for core in range(NCORES):
        b, hh = core // 2, core % 2
        cs = slice(hh * OC, (hh + 1) * OC)
        xT = np.ascontiguousarray(x[b].T).astype(NPBF16)       # [D, S]
        m = np.ascontiguousarray(
            am[b, 0, 0, :].astype(np.float32).reshape(NSK, 128).T)  # [128, NSK]
        in_maps.append({
            "xT": xT,
            "wq": np.ascontiguousarray(wqT[:, cs]),
            "wk": np.ascontiguousarray(wkT[:, cs]),
            "wv": np.ascontiguousarray(wvT[:, cs]),
            "wo": np.ascontiguousarray(woT[cs, :]),
            "mask": m,
        })
    return in_maps


def kernel(_trace=False, _trace_kwargs=None, **inputs):
    x = np.asarray(inputs["hidden_states"], dtype=np.float32)
    am = np.asarray(inputs["attention_mask"], dtype=np.float32)
    Wq = np.asarray(inputs["Wq"], dtype=np.float32)
    bq = np.asarray(inputs["bq"], dtype=np.float32)
    Aq = np.asarray(inputs["Aq"], dtype=np.float32)
    Bq = np.asarray(inputs["Bq"], dtype=np.float32)
    Wk = np.asarray(inputs["Wk"], dtype=np.float32)
    bk = np.asarray(inputs["bk"], dtype=np.float32)
    Wv = np.asarray(inputs["Wv"], dtype=np.float32)
    bv = np.asarray(inputs["bv"], dtype=np.float32)
    Av = np.asarray(inputs["Av"], dtype=np.float32)
    Bv = np.asarray(inputs["Bv"], dtype=np.float32)
    Wo = np.asarray(inputs["Wo"], dtype=np.float32)
    bo = np.asarray(inputs["bo"], dtype=np.float32)

    # The on-device kernel folds q-scaling into the weights and handles the
    # additive mask; projection biases are all-zero in this problem's
    # regime (asserted here so a violated assumption fails loudly rather
    # than silently returning wrong results).
    assert not bq.any() and not bk.any() and not bv.any(), (
        "non-zero projection biases not supported by this kernel build")

    # The additive mask is all-zeros in this problem's regime; the no-bias
    # exp is measurably faster on the ACT engine, so dispatch on the actual
    # input and keep the bias build as the general fallback.
    key = "nc" if am.any() else "nc_nomask"
    if key not in _NC_CACHE:
        _NC_CACHE[key] = _build_nc(use_mask_bias=(key == "nc"))
    nc = _NC_CACHE[key]

    in_maps = _prep_core_inputs(x, am, Wq, Aq, Bq, Wk, Wv, Av, Bv, Wo)
    res = run_bass_kernel_spmd(
        nc, in_maps, core_ids=list(range(NCORES)), trace=_trace,
        trace_kwargs=_trace_kwargs or {})
    outs = res.results

    out = np.empty((B, S, D), dtype=np.float32)
    for b in range(B):
        out[b] = (outs[2 * b]["out"].astype(np.float32)
                  + outs[2 * b + 1]["out"].astype(np.float32) + bo)
    if _trace:
        return out, res
    return out
